# revision 9
# baseline (speedup 1.0000x reference)
"""Trainium2 Bass kernel for nn_BasicRecurrentEntityEncoder.

Data-parallel over batch B=256 across 8 NeuronCores (32 batches/core).
Per core, entity rows are laid out k-major: j = k*32 + b (K padded 30->32),
giving 1024 rows = 8 chunks x 128 partitions with b = p % 32 uniform in
every chunk. State is kept both natural ([128, 8, 256], row-major) and
transposed ([d, j], for PE matmuls); the transpose is refreshed on PE each
step via identity matmuls.

Phase A: indirect-DMA gather of bf16 embedding rows (masked tokens point at
a zero row), on-chip bag-of-words sums, encT / e@W / e.keys (EK) / keys@V
precomputes; later groups gather early and precompute mid-group so the PSUM
pool rotation never blocks the scan.

Phase B (v2): the 64-step recurrence as two 4-chunk half-pipelines per step.
Per half: gate MMs + hU MMs share LDWEIGHTS (emitted adjacently, ldw-opt
merges the redundant loads), kv/eW injected via ident/selkm matmuls; the
gate sigmoid runs natively on ACT (sigmoid_and_others table also carries
relu/square/copy so only one table load ever happens); g folds into the
relu as a per-partition ACT scale (g>0 so relu(g*x) = g*relu(x)); the
update h + g*h_tld is ONE big 2x-mode tensor_tensor add per half; ||upd||^2
accumulates via DVE tensor_tensor_reduce + ACT square (split for engine
balance); rsqrt is a 2-op magic seed + a single custom fused-NR DVE op
(registered at import); normalize splits DVE tensor_scalar / ACT copy-scale.
Both halves are emitted PE-first so the tensor engine never idles and stays
HAM-warm.
"""

import os
import re
import numpy as np
import ml_dtypes

B, S, L, D, K, VOCAB = 256, 64, 16, 256, 30, 50000
NCORES = 8
BL = B // NCORES          # 32 batches per core
KH = 32                   # padded K
J = KH * BL               # 1024 rows per core
CH = 8                    # row chunks (128 partitions each)
HCH = 4                   # chunks per half-pipeline
GRP = 8                   # gather groups
SPG = S // GRP            # steps per group
NEG = -60.0               # gate logit offset for masked sentences
EPS = 1e-12

LAST_EXEC_NS = None       # set when BASS_KERNEL_TRACE=1
NSTEPS = int(os.environ.get("BK_NSTEPS", str(S)))
SKIP_GATHER = os.environ.get("BK_SKIP_GATHER", "0") == "1"
# custom DVE ops (incl. tensor_tensor_reduce) hang the device through this
# runtime — the per-NEFF DVE ucode table doesn't reach the DVE. Keep off.
USE_CUSTOM_NR = os.environ.get("BK_CUSTOM_NR", "0") == "1"

_bf16 = ml_dtypes.bfloat16

_RSQRT_NR = None


def _register_rsqrt_nr():
    """Register a custom DVE op: out = in0*(1.5 - 0.5*in1*in0^2) — one
    Newton-Raphson step for rsqrt (in0 = magic seed y0, in1 = ss)."""
    global _RSQRT_NR
    if _RSQRT_NR is not None:
        return _RSQRT_NR
    from concourse import dve_ops as _do
    from concourse.dve_spec import Spec, Src0, Src1, C0, C1, sq

    def _ref(in0, in1, c0, c1, c2):
        x = in0.astype(np.float32)
        return (c0 + c1 * in1.astype(np.float32) * (x * x)) * x

    name = "RSQRT_NR_ANT"
    for op in _do.OPS:
        if op.name == name:
            _RSQRT_NR = op
            return op
    op = _do.DveOp(
        name,
        Spec(body=(C0 + C1 * Src1 * sq(Src0)) * Src0, reference=_ref),
        subdim=False,
        uops_sha={},
    )
    _do.OPS.append(op)
    _do._SUB_OPCODE_FOR_NAME[name] = max(_do._SUB_OPCODE_FOR_NAME.values()) + 1
    _do.CUSTOM_DVE_SPECS[name] = op.spec
    for ver in ("v3", "v4"):
        try:
            op.compile(ver)
        except ValueError as e:
            m = re.search(ver + r": ([0-9a-f]+)", str(e))
            op.uops_sha[ver] = m.group(1)
            op.compile(ver)
    _RSQRT_NR = op
    return op


def _build_nc():
    import concourse.bacc as bacc
    import concourse.bass as bass
    import concourse.mybir as mybir
    from concourse import tile

    rsqrt_nr = _register_rsqrt_nr()

    f32 = mybir.dt.float32
    bf16 = mybir.dt.bfloat16
    i32 = mybir.dt.int32
    MULT = mybir.AluOpType.mult
    ADD = mybir.AluOpType.add

    nc = bacc.Bacc("TRN2", target_bir_lowering=False, debug=False,
                   num_devices=NCORES)

    # ---- DRAM parameters -------------------------------------------------
    emb = nc.dram_tensor("emb", [8192, 4 * D], bf16, kind="ExternalInput")
    idx_d = nc.dram_tensor("idx", [128, 512], mybir.dt.int16, kind="ExternalInput")
    keysT_d = nc.dram_tensor("keysT", [128, 2, J], bf16, kind="ExternalInput")
    u_d = nc.dram_tensor("u", [128, 2, D], bf16, kind="ExternalInput")
    v_d = nc.dram_tensor("v", [128, 2, D], bf16, kind="ExternalInput")
    w_d = nc.dram_tensor("w", [128, 2, D], bf16, kind="ExternalInput")
    bias_d = nc.dram_tensor("bias", [128, S], f32, kind="ExternalInput")
    selsum_d = nc.dram_tensor("selsum", [128, BL], bf16, kind="ExternalInput")
    selkm_d = nc.dram_tensor("selkm", [BL, 128], bf16, kind="ExternalInput")
    mdiag_d = nc.dram_tensor("mdiag", [128, BL], f32, kind="ExternalInput")
    ident_d = nc.dram_tensor("ident", [128, 128], bf16, kind="ExternalInput")
    y_d = nc.dram_tensor("y", [BL, K, D], f32, kind="ExternalOutput")

    with tile.TileContext(nc) as tc:
        ctxs = []

        def pool(name, bufs, space="SBUF"):
            p = tc.tile_pool(name=name, bufs=bufs, space=space)
            ctxs.append(p)
            return p.__enter__()

        persist = pool("persist", 1)
        gbuf = pool("gbuf", 2)
        work = pool("work", 2)                  # per-(step, half) scratch
        ps_pn = pool("ps_pn", 2, "PSUM")        # [128, 4, 256] f32 = 2 banks
        ps_sm = pool("ps_sm", 2, "PSUM")        # [128, <=256] f32 slots
        ps_t = pool("ps_t", 2, "PSUM")          # [128, 2, 512] bf16 = 1 bank

        # ---- persistent SBUF tensors ------------------------------------
        idx_sb = persist.tile([128, 512], mybir.dt.int16, tag="idx")
        keysT = persist.tile([128, 2, J], bf16, tag="keysT")
        u_sb = persist.tile([128, 2, D], bf16, tag="u")
        v_sb = persist.tile([128, 2, D], bf16, tag="v")
        w_sb = persist.tile([128, 2, D], bf16, tag="w")
        bias_sb = persist.tile([128, S], f32, tag="bias")
        selsum = persist.tile([128, BL], bf16, tag="selsum")
        selkm = persist.tile([BL, 128], bf16, tag="selkm")
        mdiag = persist.tile([128, BL], f32, tag="mdiag")
        ident = persist.tile([128, 128], bf16, tag="ident")
        encT = persist.tile([128, 2, S * BL], bf16, tag="encT")
        ew_all = persist.tile([BL, S * D], bf16, tag="ew")
        ekm = persist.tile([128, CH, S], f32, tag="ekm")
        kv = persist.tile([128, CH, D], bf16, tag="kv")
        h_nat = persist.tile([128, CH, D], bf16, tag="h_nat")
        hT = persist.tile([128, 2, J], bf16, tag="hT")
        hf32 = persist.tile([128, CH, D], f32, tag="hf32")

        # ---- load parameters --------------------------------------------
        nc.sync.dma_start(out=idx_sb[:], in_=idx_d.ap())
        nc.sync.dma_start(out=keysT[:], in_=keysT_d.ap())
        nc.sync.dma_start(out=u_sb[:], in_=u_d.ap())
        nc.sync.dma_start(out=v_sb[:], in_=v_d.ap())
        nc.sync.dma_start(out=w_sb[:], in_=w_d.ap())
        nc.sync.dma_start(out=bias_sb[:], in_=bias_d.ap())
        nc.sync.dma_start(out=selsum[:], in_=selsum_d.ap())
        nc.sync.dma_start(out=selkm[:], in_=selkm_d.ap())
        nc.sync.dma_start(out=mdiag[:], in_=mdiag_d.ap())
        nc.sync.dma_start(out=ident[:], in_=ident_d.ap())

        nc.vector.memset(h_nat[:], 0.0)
        nc.vector.memset(hT[:], 0.0)

        # ========== interleaved: gathers + per-group precompute + scan ====
        def emit_gather(g):
            raw = gbuf.tile([128, 4 * SPG, D], bf16, tag="raw")
            nc.gpsimd.dma_gather(
                out_ap=raw[:].rearrange("p (q k) d -> p q (k d)", k=4),
                in_ap=emb.ap(),
                idxs_ap=idx_sb[:, g * 64:(g + 1) * 64],
                num_idxs=1024, num_idxs_reg=1024, elem_size=4 * D)
            return raw

        def emit_group_sums(g, raw):
            # l-sum: raw[p, (s_in, l_hi), d] -> part[p, s_in, d]
            s02 = gbuf.tile([128, SPG, 2, D], bf16, tag="s02")
            r4 = raw[:].rearrange("p (s l) d -> p s l d", l=4)
            nc.vector.tensor_tensor(out=s02[:], in0=r4[:, :, 0:2, :],
                                    in1=r4[:, :, 2:4, :], op=ADD)
            part = gbuf.tile([128, SPG, D], bf16, tag="part")
            nc.vector.tensor_tensor(out=part[:], in0=s02[:, :, 0, :],
                                    in1=s02[:, :, 1, :], op=ADD)
            # encT[half][d, (s, b)] via PE: part.T @ selsum
            for half in range(2):
                etp = ps_sm.tile([128, SPG * BL], f32, tag="sm")
                for si in range(SPG):
                    nc.tensor.matmul(
                        out=etp[:, si * BL:(si + 1) * BL],
                        lhsT=part[:, si, half * 128:(half + 1) * 128],
                        rhs=selsum[:], start=(si == 0), stop=(si == SPG - 1))
                nc.vector.tensor_copy(
                    out=encT[:, half, g * SPG * BL:(g + 1) * SPG * BL],
                    in_=etp[:])

        def emit_group_ew(g):
            # eW[b, (s, d)] for this group, in two 4-step halves
            for hg in range(2):
                ewp = ps_pn.tile([BL, 4, D], f32, tag="pn")
                for si in range(4):
                    s = g * SPG + hg * 4 + si
                    for half in range(2):
                        nc.tensor.matmul(
                            out=ewp[:, si, :],
                            lhsT=encT[:, half, s * BL:(s + 1) * BL],
                            rhs=w_sb[:, half, :],
                            start=(half == 0 and si % 2 == 0),
                            stop=(half == 1 and si % 2 == 1))
                nc.vector.tensor_copy(
                    out=ew_all[:, (g * SPG + hg * 4) * D:
                               (g * SPG + hg * 4 + 4) * D],
                    in_=ewp[:])

        def emit_group_ek(g):
            # EK for this group -> ekm[:, :, 8g:8g+8], in two 4-chunk halves
            for cg in range(2):
                gbig = ps_pn.tile([128, 4, SPG * BL], f32, tag="pn")
                for ci in range(4):
                    c = cg * 4 + ci
                    for half in range(2):
                        nc.tensor.matmul(
                            out=gbig[:, ci, :],
                            lhsT=keysT[:, half, c * 128:(c + 1) * 128],
                            rhs=encT[:, half, g * SPG * BL:(g + 1) * SPG * BL],
                            start=(half == 0 and ci % 2 == 0),
                            stop=(half == 1 and ci % 2 == 1))
                eks = work.tile([128, 4, SPG, BL], f32, tag="ekscr")
                nc.vector.tensor_tensor(
                    out=eks[:],
                    in0=gbig[:].rearrange("p c (s b) -> p c s b", s=SPG),
                    in1=mdiag[:].unsqueeze(1).unsqueeze(1).broadcast_to(
                        [128, 4, SPG, BL]),
                    op=MULT)
                red = work.tile([128, 4, SPG], f32, tag="ekred")
                nc.vector.tensor_reduce(
                    out=red[:], in_=eks[:], axis=mybir.AxisListType.X, op=ADD)
                nc.vector.tensor_tensor(
                    out=ekm[:, cg * 4:(cg + 1) * 4, g * SPG:(g + 1) * SPG],
                    in0=red[:],
                    in1=bias_sb[:, g * SPG:(g + 1) * SPG].unsqueeze(1)
                    .broadcast_to([128, 4, SPG]),
                    op=ADD)

        # kV[p, c, d] = keys @ V (needs only keysT)
        for c in range(CH):
            kvp = ps_sm.tile([128, D], f32, tag="sm")
            for half in range(2):
                nc.tensor.matmul(out=kvp[:],
                                 lhsT=keysT[:, half, c * 128:(c + 1) * 128],
                                 rhs=v_sb[:, half, :],
                                 start=(half == 0), stop=(half == 1))
            nc.vector.tensor_copy(out=kv[:, c, :], in_=kvp[:])

        RELU = mybir.ActivationFunctionType.Relu
        SQUARE = mybir.ActivationFunctionType.Square
        COPYF = mybir.ActivationFunctionType.Copy
        SIGMOID = mybir.ActivationFunctionType.Sigmoid
        SHIFT = mybir.AluOpType.logical_shift_right
        XOR = mybir.AluOpType.bitwise_xor

        def emit_half_pe(s, H):
            """Gate + pn matmuls for half H. Returns (gps, pn) PSUM tiles."""
            c0 = H * HCH
            gps = ps_sm.tile([128, HCH, BL], f32, tag="sm")
            pn = ps_pn.tile([128, HCH, D], f32, tag="pn")
            ews = ew_all[:, s * D:(s + 1) * D]
            # gate mms first: frees hT earliest, gate chain overlaps hU
            for ci in range(HCH):
                c = c0 + ci
                for half in range(2):
                    nc.tensor.matmul(out=gps[:, ci, :],
                                     lhsT=hT[:, half, c * 128:(c + 1) * 128],
                                     rhs=encT[:, half, s * BL:(s + 1) * BL],
                                     start=(ci == 0 and half == 0),
                                     stop=(ci == HCH - 1 and half == 1))
            for i in range(2):
                nc.tensor.matmul(out=pn[:, 2 * i:2 * i + 2, :],
                                 lhsT=ident[:],
                                 rhs=kv[:, c0 + 2 * i:c0 + 2 * i + 2, :],
                                 start=True, stop=False)
            for i in range(2):
                nc.tensor.matmul(
                    out=pn[:, 2 * i:2 * i + 2, :], lhsT=selkm[:],
                    rhs=ews.unsqueeze(1).broadcast_to([BL, 2, D]),
                    start=False, stop=False)
            for ci in range(HCH):
                c = c0 + ci
                for half in range(2):
                    nc.tensor.matmul(out=pn[:, ci, :],
                                     lhsT=hT[:, half, c * 128:(c + 1) * 128],
                                     rhs=u_sb[:, half, :], start=False,
                                     stop=(half == 1 and ci % 2 == 1))
            return gps, pn

        def emit_half_vec(s, H, gps, pn):
            """Gate extract + relu(g*) + upd + ss + rsqrt + normalize +
            transpose refresh for half H."""
            last = (s == NSTEPS - 1)
            c0 = H * HCH
            vh = work.tile([128, 4, HCH], f32, tag=f"vh{H}")
            g_, ss = vh[:, 0, :], vh[:, 1, :]
            ny, nw = vh[:, 2, :], vh[:, 3, :]
            gm = work.tile([128, HCH, BL], f32, tag=f"gm{H}")
            # gate: g = sigmoid(sum_b gps*mdiag + ek)
            nc.vector.tensor_tensor(
                out=gm[:], in0=gps[:],
                in1=mdiag[:].unsqueeze(1).broadcast_to([128, HCH, BL]),
                op=MULT)
            nc.vector.tensor_reduce(out=ny, in_=gm[:],
                                    axis=mybir.AxisListType.X, op=ADD)
            nc.vector.tensor_tensor(out=ny, in0=ny,
                                    in1=ekm[:, c0:c0 + HCH, s], op=ADD)
            nc.scalar.activation(g_, ny, SIGMOID)
            # h_tld_scaled = relu(g * pn)  (g > 0)
            h_tld = work.tile([128, HCH, D], bf16, tag=f"htld{H}")
            for ci in range(HCH):
                nc.scalar.activation(h_tld[:, ci, :], pn[:, ci, :], RELU,
                                     scale=g_[:, ci:ci + 1])
            # upd = h_tld_scaled + h_nat : one big 2x TT
            upd = work.tile([128, HCH, D], bf16, tag=f"upd{H}")
            nc.vector.tensor_tensor(out=upd[:], in0=h_tld[:],
                                    in1=h_nat[:, c0:c0 + HCH, :], op=ADD)
            # ss[c] = ||upd_c||^2 : 2 DVE ttr + 2 ACT square
            junk = work.tile([128, HCH, D], bf16, tag=f"junk{H}")
            for ci in range(HCH):
                if ci % 2 == 0:
                    nc.vector.scalar_tensor_tensor(
                        out=junk[:, ci, :], in0=upd[:, ci, :], scalar=1.0,
                        in1=upd[:, ci, :], op0=MULT, op1=MULT,
                        accum_out=ss[:, ci:ci + 1])
                else:
                    nc.scalar.activation(junk[:, ci, :], upd[:, ci, :],
                                         SQUARE, accum_out=ss[:, ci:ci + 1])
            # nw = rsqrt(ss): magic seed (2 ops) + fused NR (custom DVE op)
            nc.vector.tensor_scalar(
                out=ny.bitcast(i32), in0=ss.bitcast(i32), scalar1=1,
                scalar2=-1, op0=SHIFT, op1=XOR)
            nc.vector.tensor_scalar(
                out=ny.bitcast(i32), in0=ny.bitcast(i32),
                scalar1=0x5f3759e0, scalar2=None, op0=ADD)
            if USE_CUSTOM_NR:
                nc.vector._custom_dve(rsqrt_nr, out=nw, in0=ny, in1=ss,
                                      s0=1.5, s1=-0.5)
            else:
                nc.vector.tensor_tensor(out=nw, in0=ss, in1=ny, op=MULT)
                nc.vector.tensor_tensor(out=nw, in0=nw, in1=ny, op=MULT)
                nc.vector.tensor_scalar(out=nw, in0=nw, scalar1=-0.5,
                                        scalar2=1.5, op0=MULT, op1=ADD)
                nc.vector.tensor_tensor(out=nw, in0=ny, in1=nw, op=MULT)
            # normalize: h_nat = nw * upd (2 DVE TS + 2 ACT copy-scale)
            if not last:
                for ci in range(HCH):
                    c = c0 + ci
                    if ci % 2 == 0:
                        nc.vector.tensor_scalar_mul(
                            out=h_nat[:, c, :], in0=upd[:, ci, :],
                            scalar1=nw[:, ci:ci + 1])
                    else:
                        nc.scalar.activation(h_nat[:, c, :], upd[:, ci, :],
                                             COPYF, scale=nw[:, ci:ci + 1])
            else:
                for ci in range(HCH):
                    nc.vector.tensor_scalar_mul(
                        out=hf32[:, c0 + ci, :], in0=upd[:, ci, :],
                        scalar1=nw[:, ci:ci + 1])

        def emit_half_tail(s, H):
            """Transpose refresh + hT copy for half H (skipped on last)."""
            c0 = H * HCH
            pt = ps_t.tile([128, 2, HCH * 128], bf16, tag="pt")
            for half in range(2):
                for ci in range(HCH):
                    nc.tensor.transpose(
                        out=pt[:, half, ci * 128:(ci + 1) * 128],
                        in_=h_nat[:, c0 + ci, half * 128:(half + 1) * 128],
                        identity=ident[:])
            dst = hT[:, :, c0 * 128:(c0 + HCH) * 128]
            if H == 0:
                nc.vector.tensor_copy(out=dst, in_=pt[:])
            else:
                nc.scalar.copy(out=dst, in_=pt[:])

        def scan_step(s):
            last = (s == NSTEPS - 1)
            gps0, pn0 = emit_half_pe(s, 0)
            gps1, pn1 = emit_half_pe(s, 1)
            emit_half_vec(s, 0, gps0, pn0)
            if not last:
                emit_half_tail(s, 0)
            emit_half_vec(s, 1, gps1, pn1)
            if not last:
                emit_half_tail(s, 1)

        if not SKIP_GATHER:
            # group 0 up front; later groups gather early / precompute
            # mid-group so the PSUM pool rotation never blocks step 0
            raws = {0: emit_gather(0)}
            emit_group_sums(0, raws.pop(0))
            emit_group_ew(0)
            emit_group_ek(0)
            for g in range(GRP):
                for si in range(SPG):
                    s = g * SPG + si
                    if s >= NSTEPS:
                        continue
                    if si == 1 and g + 1 < GRP:
                        raws[g + 1] = emit_gather(g + 1)
                    if si == 3 and g + 1 < GRP:
                        emit_group_sums(g + 1, raws.pop(g + 1))
                    if si == 4 and g + 1 < GRP:
                        emit_group_ew(g + 1)
                    if si == 5 and g + 1 < GRP:
                        emit_group_ek(g + 1)
                    scan_step(s)
        else:
            nc.vector.memset(encT[:], 0.0)
            nc.vector.memset(ew_all[:], 0.0)
            nc.vector.memset(ekm[:], 0.0)
            for s in range(NSTEPS):
                scan_step(s)

        if NSTEPS == 0:
            nc.vector.memset(hf32[:], 0.0)
        # ---- output: y[b, k, d] <- hf32[(k%4)*32+b, k//4, d] -------------
        y_main = y_d.ap()[:, 0:28, :].rearrange("b (kh kl) d -> b kl kh d",
                                                kl=4)
        for klo in range(4):
            nc.sync.dma_start(out=y_main[:, klo, :, :],
                              in_=hf32[klo * 32:(klo + 1) * 32, 0:7, :])
        nc.sync.dma_start(out=y_d.ap()[:, 28, :],
                          in_=hf32[0:32, 7, :])
        nc.sync.dma_start(out=y_d.ap()[:, 29, :],
                          in_=hf32[32:64, 7, :])

        for p in reversed(ctxs):
            p.__exit__(None, None, None)

    nc.compile()
    return nc


def _host_prep(prgrph, prgrph_mask, keys, embedding_matrix, U, V, W):
    """Build per-core input maps."""
    prg = np.asarray(prgrph).astype(np.int64)
    msk = np.asarray(prgrph_mask).astype(bool)
    keys = np.asarray(keys, dtype=np.float32)
    embm = np.asarray(embedding_matrix, dtype=np.float32)
    U = np.asarray(U, dtype=np.float32)
    V = np.asarray(V, dtype=np.float32)
    W = np.asarray(W, dtype=np.float32)

    emb_bf = embm.astype(_bf16)

    def halves(m):      # [256, 256] -> [128, 2, 256] bf16
        return np.ascontiguousarray(
            m.reshape(2, 128, D).swapaxes(0, 1).astype(_bf16))

    u_h, v_h, w_h = halves(U), halves(V), halves(W)

    ident = np.eye(128, dtype=_bf16)
    selsum = np.zeros((128, BL), dtype=_bf16)
    p_ar = np.arange(128)
    selsum[p_ar, p_ar % 32] = 1
    selkm = np.ascontiguousarray(selsum.T)
    mdiag = selsum.astype(np.float32)

    # token index layout: flat slot i=q*128+p, p=(l%4)*32+b, q=g*32+s_in*4+l//4
    tok = np.where(msk, prg, VOCAB).astype(np.int64)   # [B, S, L]
    sent_ok = msk.any(-1)                              # [B, S]

    in_maps = []
    for m in range(NCORES):
        b0 = m * BL
        t = tok[b0:b0 + BL]                            # [32, 64, 16]
        # quad dedup: one table row = the 4 l_hi embeddings of (b, s, l_lo)
        quads = t.reshape(BL, S, 4, 4).transpose(0, 1, 3, 2)   # [b, s, l_lo, l_hi]
        qflat = np.ascontiguousarray(quads.reshape(-1, 4))
        uniq, inv = np.unique(qflat, axis=0, return_inverse=True)
        n_u = len(uniq)
        assert n_u <= 8192, f"unique quad overflow: {n_u}"
        emb_core = np.zeros((8192, 4, D), dtype=_bf16)
        safe = np.minimum(uniq, VOCAB)                  # VOCAB -> zero row
        ext = np.vstack([emb_bf, np.zeros((1, D), _bf16)])
        emb_core[:n_u] = ext[safe]
        emb_core = emb_core.reshape(8192, 4 * D)
        inv = inv.reshape(BL, S, 4)                     # [b, s, l_lo]
        # flat slot i = q*128 + p, p = l_lo*32 + b, q = s_in (per group)
        idx = np.zeros((128, 64), dtype=np.int16)       # [p, g*8+s_in]
        s_idx = np.arange(S)
        g_ar, si_ar = s_idx // SPG, s_idx % SPG
        for llo in range(4):
            p = llo * 32 + np.arange(BL)
            q = g_ar * 8 + si_ar
            idx[p[:, None], q[None, :]] = inv[:, :, llo].astype(np.int16)
        # wrap flat order i=q*128+p into [16, n/16] gather layout per group
        cols = []
        for g in range(GRP):
            flat = idx[:, g * 8:(g + 1) * 8].T.reshape(-1)   # i = s_in*128+p
            cols.append(flat.reshape(64, 16).T)
        idx16 = np.ascontiguousarray(np.tile(np.concatenate(cols, axis=1), (8, 1)))
        kT = np.zeros((D, J), dtype=_bf16)
        kloc = np.transpose(keys[b0:b0 + BL], (2, 1, 0))   # [D, K, BL]
        kT[:, :K * BL] = kloc.reshape(D, K * BL)[:, :]
        # j = k*32 + b -> reshape (K, BL) row-major matches k*32+b
        keysT_h = np.ascontiguousarray(kT.reshape(2, 128, J).swapaxes(0, 1))
        bias = np.zeros((128, S), dtype=np.float32)
        ok = sent_ok[b0:b0 + BL]                       # [32, 64]
        bias[:, :] = np.where(ok, 0.0, NEG)[np.arange(128) % 32, :]
        in_maps.append({
            "emb": emb_core, "idx": idx16, "keysT": keysT_h,
            "u": u_h, "v": v_h, "w": w_h, "bias": bias,
            "selsum": selsum, "selkm": selkm, "mdiag": mdiag,
            "ident": ident,
        })
    return in_maps


def _patch_ldw_opt():
    # flip walrus's --enable-ldw-opt for our own compile invocation:
    # gate-mm/hU-mm pairs share lhsT, so merging redundant LDWEIGHTS
    # saves PE issue slots
    import concourse.bass_utils as _bu
    if getattr(_bu, "_bk_ldw_patched", False):
        return
    _orig_rc = _bu.run_command

    def _rc(argv, **kw):
        argv = ["--enable-ldw-opt=true" if a == "--enable-ldw-opt=false"
                else a for a in argv]
        return _orig_rc(argv, **kw)

    _bu.run_command = _rc
    _bu._bk_ldw_patched = True


def kernel(**inputs):
    global LAST_EXEC_NS
    from concourse.bass_utils import run_bass_kernel_spmd
    if os.environ.get("BK_LDW_OPT", "0") == "1":
        _patch_ldw_opt()

    trace = os.environ.get("BASS_KERNEL_TRACE", "0") == "1"
    if trace:
        try:
            import sys, types, contextlib

            if "antenv.axon_hooks" not in sys.modules:
                mod = types.ModuleType("antenv.axon_hooks")
                _h = [None]
                mod.set_axon_ntff_profile_hook = lambda h: _h.__setitem__(0, h)
                mod.get_axon_ntff_profile_hook = lambda: _h[0]
                sys.modules["antenv.axon_hooks"] = mod
                import antenv
                antenv.axon_hooks = mod
                from trn_agent_boot.trn_boot import _ntff_profile_via_ctypes
                mod.set_axon_ntff_profile_hook(
                    _ntff_profile_via_ctypes("/opt/axon/libaxon_pjrt.so"))
        except Exception as e:
            print("trace hook unavailable:", e)
            trace = False

    nc = _build_nc()
    in_maps = _host_prep(**inputs)
    res = run_bass_kernel_spmd(nc, in_maps, list(range(NCORES)), trace=trace)
    if trace:
        LAST_EXEC_NS = res.exec_time_ns
    out = np.concatenate([res.results[m]["y"] for m in range(NCORES)], axis=0)
    return out.astype(np.float32)
